# revision 25
# baseline (speedup 1.0000x reference)
"""Trainium2 Bass kernel for nn_CausalPatternDetector.

Computes mean |corr(x[1:, i], x[:-1, j])| over i != j for x [32768, 1024] f32.

Strategy (8 NeuronCores, 2i x 2j x 2t sharding, fp8 DoubleRow):
  - Host quantizes x to fp8 e4m3 and packs per-core operands in the
    [128, ksub=2, free] DoubleRow layout. Core c = 4*ib + 2*jb + t computes
    the cov partial for i-block [512ib, +512) x j-block [512jb, +512) over
    time-half t. lhsT (xc i-cols, ordered [partner's 256 | own 256], own
    half pre-scaled by 0.25) and rhs (xl j-cols) are merged into one DRAM
    tensor with 8KB rows (16 x 1MB DMAs, 16.8MB/core, ~373 GB/s).
  - Pass 1 (partner's two 128-row chunks) accumulates over all 64 kpairs;
    pass 2 (own chunks) trails (last 5 groups after the RS trigger) so the
    pairwise bf16 ReduceScatter of the pass-1 partial (x0.25, seg-selected
    by a host 0/0.25 flag) can overlap it. recv + own psum = centered cov.
  - Normalization is a pure rescaling (0.1% of FLOPs): the host computes
    exact per-feature sums/sums-of-squares of the fp8 data and ships the
    combined weight tile wgt = 4*mask*rsqrt(nl_j)*rsqrt(nc_i) and the
    mean-outer tile mo/4 per core (bf16, 256KB each). This removes the
    8-way stats AllReduce (the dominant cross-core-skew amplifier); the
    only collective left is the pairwise RS, triggered asynchronously at
    kernel start (its ~38us CC setup runs under the load phase).
  - Tail: |own*0.25 - mo/4 + recv| * (4*wgt) == |cov - mo| * rsqrt-outer,
    abs-reduce -> [128,2]; host sums, scales by 1/(F(F-1)).
"""

import numpy as np
import ml_dtypes

import concourse.bass as bass
import concourse.mybir as mybir
import concourse.tile as tile
from concourse import bacc
from concourse.bass_utils import run_bass_kernel_spmd

P = 128
F = 1024
T = 32768
H = T // 2           # 16384 contraction rows per half
KP = 64              # k-pairs per core (each = 256 rows)
NG = 16              # groups of 4 kpairs
NGI = 11             # pass-2 groups before the RS trigger
NCORES = 8
N = float(T - 1)     # 32767 pair count
F8 = mybir.dt.float8e4
F32 = mybir.dt.float32
BF16 = mybir.dt.bfloat16
NP_F8 = ml_dtypes.float8_e4m3

_CACHE = {}


def _build():
    nc = bacc.Bacc("TRN2", target_bir_lowering=False, debug=False,
                   num_devices=NCORES)

    # merged per-group rows: [lh 4096B | rh 4096B]
    xin = nc.dram_tensor("xin", [NG * P, 8192], F8, kind="ExternalInput")
    wgt_in = nc.dram_tensor("wgt", [P, 2, 512], BF16, kind="ExternalInput")
    mo_in = nc.dram_tensor("mo", [P, 2, 512], BF16, kind="ExternalInput")
    zc = nc.dram_tensor("zc", [P, 2], F32, kind="ExternalInput")
    out = nc.dram_tensor("out", [P, 2], F32, kind="ExternalOutput")

    add = mybir.AluOpType.add
    mult = mybir.AluOpType.mult
    DR = mybir.MatmulPerfMode.DoubleRow

    with tile.TileContext(nc) as tc:
        with (
            tc.tile_pool(name="dram", bufs=1, space="DRAM") as dram,
            tc.tile_pool(name="xp", bufs=1) as xp,
            tc.tile_pool(name="normp", bufs=1) as normp,
            tc.tile_pool(name="psum", bufs=8, space="PSUM") as psum,
        ):
            rs_in = dram.tile([4 * P, 512], BF16)
            rs_out = dram.tile([2 * P, 512], BF16)
            dum_in = dram.tile([2, 512], BF16)
            dum_out = dram.tile([1, 512], BF16)

            # ---- loads: tile [P, 2(lh/rh), 4(ki), 2(ksub), 512] ----
            # bulk loads ride the sync queue's rings only, so the scalar/
            # gpsimd rings stay empty for small latency-critical DMAs
            xg = []

            def load_group(g, split=False):
                t = xp.tile([P, 2, 4, 2, 512], F8, name=f"x{g}", tag=f"x{g}")
                if split:
                    for h in range(2):
                        for ki in range(4):
                            nc.sync.dma_start(
                                t[:, h, ki],
                                xin[P * g: P * g + P,
                                    4096 * h + 1024 * ki:
                                    4096 * h + 1024 * ki + 1024])
                else:
                    nc.sync.dma_start(t[:], xin[P * g: P * g + P, :])
                xg.append(t)

            # warm the CC path: tiny pairwise RS triggered before anything
            # else so the ~13us collective setup runs under the load phase
            dumt = normp.tile([2, 512], BF16)
            nc.gpsimd.memset(dumt[:], 0.0)
            nc.gpsimd.dma_start(dum_in[:], dumt[:])
            nc.gpsimd.collective_compute(
                "ReduceScatter", add,
                replica_groups=[[2 * i, 2 * i + 1] for i in range(4)],
                ins=[dum_in.opt()], outs=[dum_out.opt()])
            load_group(0, split=True)
            ones8 = normp.tile([P, 2, 16], F8)
            nc.gpsimd.memset(ones8[:], 1.0)
            warm = normp.tile([P, 2, 512], F8)
            nc.vector.memset(warm[:], 0.0)
            psw = psum.tile([16, 512], F32, name="psw", tag="ps")
            for i in range(6):
                nc.tensor.matmul(psw[:], ones8[:], warm[:],
                                 start=(i == 0), stop=(i == 5),
                                 perf_mode=DR)
            wgt = normp.tile([P, 2, 512], BF16)
            nc.gpsimd.dma_start(wgt[:], wgt_in[:])
            mo = normp.tile([P, 2, 512], BF16)
            nc.gpsimd.dma_start(mo[:], mo_in[:])
            zc_t = normp.tile([P, 2], F32)
            nc.gpsimd.dma_start(zc_t[:], zc[:])
            for g in range(1, NG):
                load_group(g)

            # ---- matmuls: pass1 = lh cols [0:256] (partner rows, all kp),
            #      pass2 = cols [256:512] (own rows, g<NGI inline) ----
            ps1 = [psum.tile([P, 512], F32, name=f"p1{u}", tag="ps")
                   for u in range(2)]
            ps2 = [psum.tile([P, 512], F32, name=f"p2{u}", tag="ps")
                   for u in range(2)]

            def mm(bank, u, g, ki, kp, base):
                nc.tensor.matmul(
                    bank[:],
                    xg[g][:, 0, ki, :, base + 128 * u: base + 128 * u + 128],
                    xg[g][:, 1, ki],
                    start=(kp == 0), stop=(kp == KP - 1), perf_mode=DR)

            for g in range(NG):
                for ki in range(4):
                    kp = 4 * g + ki
                    for u in range(2):
                        mm(ps1[u], u, g, ki, kp, 0)
                    if g < NGI:
                        for u in range(2):
                            mm(ps2[u], u, g, ki, kp, 256)

            # ---- pass-1 flush (x 0.25/0 seg flags) + bf16 pairwise RS ----
            ot = normp.tile([P, 2, 2, 512], BF16)
            for s in range(2):
                for u in range(2):
                    nc.vector.tensor_scalar(ot[:, s, u], ps1[u][:],
                                            zc_t[:, s: s + 1], None, mult)
                nc.scalar.dma_start(
                    rs_in[256 * s: 256 * s + 256, :].rearrange(
                        "(u p) n -> p u n", p=P), ot[:, s])
            nc.gpsimd.collective_compute(
                "ReduceScatter", add,
                replica_groups=[[2 * i, 2 * i + 1] for i in range(4)],
                ins=[rs_in.opt()], outs=[rs_out.opt()])

            # ---- trailing pass-2 matmuls overlap the RS ----
            for g in range(NGI, NG):
                for ki in range(4):
                    kp = 4 * g + ki
                    for u in range(2):
                        mm(ps2[u], u, g, ki, kp, 256)

            # ---- tail: covm = 0.25*own - mo/4 ; covf = covm + recv ----
            covm = normp.tile([P, 2, 512], BF16)
            for u in range(2):
                nc.vector.tensor_sub(covm[:, u], ps2[u][:], mo[:, u])
            recv = normp.tile([P, 2, 512], BF16)
            nc.scalar.dma_start(
                recv[:], rs_out[:, :].rearrange("(u p) n -> p u n", p=P))
            covf = normp.tile([P, 2, 512], BF16)
            nc.vector.tensor_add(covf[:], covm[:], recv[:])
            nc.vector.tensor_mul(covf[:], covf[:], wgt[:])
            rsum = normp.tile([P, 2], F32)
            nc.vector.tensor_reduce(rsum[:], covf[:], mybir.AxisListType.X,
                                    add, apply_absolute_value=True)
            nc.scalar.dma_start(out[:], rsum[:])

    nc.compile()
    return nc


def _in_maps(x: np.ndarray):
    x8 = np.ascontiguousarray(x, dtype=np.float32).astype(NP_F8)
    xf = x8.astype(np.float32)
    # exact per-feature stats of the fp8 data over the lag windows
    S_all = xf.sum(axis=0)
    Q_all = (xf * xf).sum(axis=0)
    Sl = S_all - xf[T - 1]
    Ql = Q_all - xf[T - 1] * xf[T - 1]
    Sc = S_all - xf[0]
    Qc = Q_all - xf[0] * xf[0]
    nl = Ql - Sl * Sl / N
    ncv = Qc - Sc * Sc / N
    rsl = 1.0 / np.sqrt(nl)
    rsc = 1.0 / np.sqrt(ncv)
    maps = []
    for c in range(NCORES):
        ib, jb, t = c >> 2, (c >> 1) & 1, c & 1
        lo = H * t
        hi = min(lo + H, T - 1)
        n = hi - lo
        xl = np.zeros((H, 512), dtype=NP_F8)
        xl[:n] = x8[lo:hi, 512 * jb: 512 * jb + 512]
        # lh cols: [partner's 256 | own 256 scaled by 0.25]
        xc = np.zeros((H, 512), dtype=NP_F8)
        own0 = 512 * ib + 256 * t
        par0 = 512 * ib + 256 * (1 - t)
        xc[:n, 0:256] = x8[lo + 1: hi + 1, par0: par0 + 256]
        xc[:n, 256:512] = (x8[lo + 1: hi + 1, own0: own0 + 256]
                           .astype(np.float32) * 0.25).astype(NP_F8)
        xl_k = xl.reshape(KP, 2, P, 512)
        xc_k = xc.reshape(KP, 2, P, 512)
        rha = xl_k.reshape(NG, 4, 2, P, 512).transpose(0, 3, 1, 2, 4)
        lha = xc_k.reshape(NG, 4, 2, P, 512).transpose(0, 3, 1, 2, 4)
        xina = np.concatenate(
            [lha.reshape(NG, P, 4096), rha.reshape(NG, P, 4096)],
            axis=2).reshape(NG * P, 8192)
        # owned rows: global i = own0 + 128*s + p ; cols j = 512*jb + jj
        irows = own0 + (np.arange(2)[:, None] * 128
                        + np.arange(P)[None, :]).reshape(2, P)  # [2, P]
        jcols = 512 * jb + np.arange(512)                       # [512]
        wgt = (4.0 * rsc[irows][:, :, None]
               * rsl[jcols][None, None, :])                     # [2, P, 512]
        moq = (0.25 / N * Sc[irows][:, :, None]
               * Sl[jcols][None, None, :])
        # mask the true diagonal
        dmask = irows[:, :, None] == jcols[None, None, :]
        wgt = np.where(dmask, 0.0, wgt)
        wgt_t = np.ascontiguousarray(
            wgt.transpose(1, 0, 2)).astype(ml_dtypes.bfloat16)
        mo_t = np.ascontiguousarray(
            moq.transpose(1, 0, 2)).astype(ml_dtypes.bfloat16)
        zcv = np.zeros((P, 2), dtype=np.float32)
        zcv[:, 1 - t] = 0.25                       # my pass1 -> partner seg
        maps.append({"xin": np.ascontiguousarray(xina), "wgt": wgt_t,
                     "mo": mo_t, "zc": zcv})
    return maps


def kernel(x: np.ndarray, _trace: bool = False, **_):
    if "nc" not in _CACHE:
        _CACHE["nc"] = _build()
    nc = _CACHE["nc"]
    res = run_bass_kernel_spmd(nc, _in_maps(x), core_ids=list(range(NCORES)),
                               trace=_trace)
    total = np.float64(0.0)
    for k in range(NCORES):
        total += np.float64(res.results[k]["out"].astype(np.float64).sum())
    _CACHE["last_results"] = res
    return np.asarray(total / (F * (F - 1.0)), dtype=np.float32)


# revision 29
# speedup vs baseline: 1.0487x; 1.0487x over previous
"""Trainium2 Bass kernel for nn_CausalPatternDetector.

Computes mean |corr(x[1:, i], x[:-1, j])| over i != j for x [32768, 1024] f32.

Strategy (8 NeuronCores, 2i x 2j x 2t sharding, fp8 DoubleRow):
  - Host quantizes x to fp8 e4m3 and packs per-core operands in the
    [128, ksub=2, free] DoubleRow layout. Core c = 4*ib + 2*jb + t computes
    the cov partial for i-block [512ib, +512) x j-block [512jb, +512) over
    time-half t. lhsT (xc i-cols, ordered [partner's 256 | own 256], own
    half pre-scaled by 0.25) and rhs (xl j-cols) are merged into one DRAM
    tensor with 8KB rows (16 x 1MB DMAs, 16.8MB/core, ~373 GB/s).
  - Pass 1 (partner's two 128-row chunks) accumulates over all 64 kpairs;
    pass 2 (own chunks) trails (last 5 groups after the RS trigger) so the
    pairwise bf16 ReduceScatter of the pass-1 partial (x0.25, seg-selected
    by a host 0/0.25 flag) can overlap it. recv + own psum = centered cov.
  - Normalization is a pure rescaling (0.1% of FLOPs): the host computes
    exact per-feature sums/sums-of-squares of the fp8 data and ships the
    combined weight tile wgt = 4*mask*rsqrt(nl_j)*rsqrt(nc_i) and the
    mean-outer tile mo/4 per core (bf16, 256KB each). This removes the
    8-way stats AllReduce (the dominant cross-core-skew amplifier); the
    only collective left is the pairwise RS, triggered asynchronously at
    kernel start (its ~38us CC setup runs under the load phase).
  - Tail: |own*0.25 - mo/4 + recv| * (4*wgt) == |cov - mo| * rsqrt-outer,
    abs-reduce -> [128,2]; host sums, scales by 1/(F(F-1)).
"""

import numpy as np
import ml_dtypes

import concourse.bass as bass
import concourse.mybir as mybir
import concourse.tile as tile
from concourse import bacc
from concourse.bass_utils import run_bass_kernel_spmd

P = 128
F = 1024
T = 32768
H = T // 2           # 16384 contraction rows per half
KP = 64              # k-pairs per core (each = 256 rows)
NG = 16              # groups of 4 kpairs
NGI = 11             # pass-2 groups before the RS trigger
NCORES = 8
N = float(T - 1)     # 32767 pair count
F8 = mybir.dt.float8e4
F32 = mybir.dt.float32
BF16 = mybir.dt.bfloat16
NP_F8 = ml_dtypes.float8_e4m3

_CACHE = {}


def _build():
    nc = bacc.Bacc("TRN2", target_bir_lowering=False, debug=False,
                   num_devices=NCORES)

    # merged per-group rows: [lh 4096B | rh 4096B]
    xin = nc.dram_tensor("xin", [NG * P, 8192], F8, kind="ExternalInput")
    wgt_in = nc.dram_tensor("wgt", [P, 2, 512], BF16, kind="ExternalInput")
    mo_in = nc.dram_tensor("mo", [P, 2, 512], BF16, kind="ExternalInput")
    out = nc.dram_tensor("out", [P, 2], F32, kind="ExternalOutput")

    add = mybir.AluOpType.add
    mult = mybir.AluOpType.mult
    DR = mybir.MatmulPerfMode.DoubleRow

    with tile.TileContext(nc) as tc:
        with (
            tc.tile_pool(name="dram", bufs=1, space="DRAM") as dram,
            tc.tile_pool(name="xp", bufs=1) as xp,
            tc.tile_pool(name="normp", bufs=1) as normp,
            tc.tile_pool(name="psum", bufs=8, space="PSUM") as psum,
        ):
            rs_in = dram.tile([2 * P, 512], BF16)
            rs_out = dram.tile([4 * P, 512], BF16)
            dum_in = dram.tile([1, 512], BF16)
            dum_out = dram.tile([2, 512], BF16)

            # ---- loads: tile [P, 2(lh/rh), 4(ki), 2(ksub), 512] ----
            # bulk loads ride the sync queue's rings only, so the scalar/
            # gpsimd rings stay empty for small latency-critical DMAs
            xg = []

            def load_group(g, split=False):
                t = xp.tile([P, 2, 4, 2, 512], F8, name=f"x{g}", tag=f"x{g}")
                if split:
                    for h in range(2):
                        for ki in range(4):
                            nc.sync.dma_start(
                                t[:, h, ki],
                                xin[P * g: P * g + P,
                                    4096 * h + 1024 * ki:
                                    4096 * h + 1024 * ki + 1024])
                else:
                    nc.sync.dma_start(t[:], xin[P * g: P * g + P, :])
                xg.append(t)

            # warm the CC path: tiny pairwise RS triggered before anything
            # else so the ~13us collective setup runs under the load phase
            dumt = normp.tile([1, 512], BF16)
            nc.gpsimd.memset(dumt[:], 0.0)
            nc.gpsimd.dma_start(dum_in[:], dumt[:])
            nc.gpsimd.collective_compute(
                "AllGather", mybir.AluOpType.bypass,
                replica_groups=[[2 * i, 2 * i + 1] for i in range(4)],
                ins=[dum_in.opt()], outs=[dum_out.opt()])
            load_group(0, split=True)
            ones8 = normp.tile([P, 2, 16], F8)
            nc.gpsimd.memset(ones8[:], 1.0)
            warm = normp.tile([P, 2, 512], F8)
            nc.vector.memset(warm[:], 0.0)
            psw = psum.tile([16, 512], F32, name="psw", tag="ps")
            for i in range(6):
                nc.tensor.matmul(psw[:], ones8[:], warm[:],
                                 start=(i == 0), stop=(i == 5),
                                 perf_mode=DR)
            wgt = normp.tile([P, 2, 512], BF16)
            nc.gpsimd.dma_start(wgt[:], wgt_in[:])
            mo = normp.tile([P, 2, 512], BF16)
            nc.gpsimd.dma_start(mo[:], mo_in[:])
            for g in range(1, NG):
                load_group(g)

            # ---- matmuls: pass1 = lh cols [0:256] (partner rows, all kp),
            #      pass2 = cols [256:512] (own rows, g<NGI inline) ----
            ps1 = [psum.tile([P, 512], F32, name=f"p1{u}", tag="ps")
                   for u in range(2)]
            ps2 = [psum.tile([P, 512], F32, name=f"p2{u}", tag="ps")
                   for u in range(2)]

            def mm(bank, u, g, ki, kp, base):
                nc.tensor.matmul(
                    bank[:],
                    xg[g][:, 0, ki, :, base + 128 * u: base + 128 * u + 128],
                    xg[g][:, 1, ki],
                    start=(kp == 0), stop=(kp == KP - 1), perf_mode=DR)

            for g in range(NG):
                for ki in range(4):
                    kp = 4 * g + ki
                    for u in range(2):
                        mm(ps1[u], u, g, ki, kp, 0)
                    if g < NGI:
                        for u in range(2):
                            mm(ps2[u], u, g, ki, kp, 256)

            # ---- pass-1 flush (x 0.25) duplicated into both blocks,
            #      exchanged via fp8 AllToAll (pure transport, no CC add);
            #      recv[0]+recv[1] = own+partner regardless of rank order --
            ot = normp.tile([P, 2, 512], BF16)
            for u in range(2):
                nc.scalar.mul(ot[:, u], ps1[u][:], 0.25)
            nc.scalar.dma_start(
                rs_in[:, :].rearrange("(u p) n -> p u n", p=P), ot[:])
            nc.gpsimd.collective_compute(
                "AllGather", mybir.AluOpType.bypass,
                replica_groups=[[2 * i, 2 * i + 1] for i in range(4)],
                ins=[rs_in.opt()], outs=[rs_out.opt()])

            # ---- trailing pass-2 matmuls overlap the RS ----
            for g in range(NGI, NG):
                for ki in range(4):
                    kp = 4 * g + ki
                    for u in range(2):
                        mm(ps2[u], u, g, ki, kp, 256)

            # ---- tail: covm = 0.25*own - mo/4 - ot_own ; + recv sum ----
            covm = normp.tile([P, 2, 512], BF16)
            for u in range(2):
                nc.vector.tensor_sub(covm[:, u], ps2[u][:], mo[:, u])
            nc.vector.tensor_sub(covm[:], covm[:], ot[:])
            recv = normp.tile([P, 2, 2, 512], BF16)
            nc.scalar.dma_start(
                recv[:], rs_out[:, :].rearrange("(s u p) n -> p s u n",
                                                s=2, u=2, p=P))
            rsb = normp.tile([P, 2, 512], BF16)
            nc.vector.tensor_add(rsb[:], recv[:, 0], recv[:, 1])
            covf = normp.tile([P, 2, 512], BF16)
            nc.vector.tensor_add(covf[:], covm[:], rsb[:])
            nc.vector.tensor_mul(covf[:], covf[:], wgt[:])
            rsum = normp.tile([P, 2], F32)
            nc.vector.tensor_reduce(rsum[:], covf[:], mybir.AxisListType.X,
                                    add, apply_absolute_value=True)
            nc.scalar.dma_start(out[:], rsum[:])

    nc.compile()
    return nc


def _in_maps(x: np.ndarray):
    x8 = np.ascontiguousarray(x, dtype=np.float32).astype(NP_F8)
    xf = x8.astype(np.float32)
    # exact per-feature stats of the fp8 data over the lag windows
    S_all = xf.sum(axis=0)
    Q_all = (xf * xf).sum(axis=0)
    Sl = S_all - xf[T - 1]
    Ql = Q_all - xf[T - 1] * xf[T - 1]
    Sc = S_all - xf[0]
    Qc = Q_all - xf[0] * xf[0]
    nl = Ql - Sl * Sl / N
    ncv = Qc - Sc * Sc / N
    rsl = 1.0 / np.sqrt(nl)
    rsc = 1.0 / np.sqrt(ncv)
    maps = []
    for c in range(NCORES):
        ib, jb, t = c >> 2, (c >> 1) & 1, c & 1
        lo = H * t
        hi = min(lo + H, T - 1)
        n = hi - lo
        xl = np.zeros((H, 512), dtype=NP_F8)
        xl[:n] = x8[lo:hi, 512 * jb: 512 * jb + 512]
        # lh cols: [partner's 256 | own 256 scaled by 0.25]
        xc = np.zeros((H, 512), dtype=NP_F8)
        own0 = 512 * ib + 256 * t
        par0 = 512 * ib + 256 * (1 - t)
        xc[:n, 0:256] = x8[lo + 1: hi + 1, par0: par0 + 256]
        xc[:n, 256:512] = (x8[lo + 1: hi + 1, own0: own0 + 256]
                           .astype(np.float32) * 0.25).astype(NP_F8)
        xl_k = xl.reshape(KP, 2, P, 512)
        xc_k = xc.reshape(KP, 2, P, 512)
        rha = xl_k.reshape(NG, 4, 2, P, 512).transpose(0, 3, 1, 2, 4)
        lha = xc_k.reshape(NG, 4, 2, P, 512).transpose(0, 3, 1, 2, 4)
        xina = np.concatenate(
            [lha.reshape(NG, P, 4096), rha.reshape(NG, P, 4096)],
            axis=2).reshape(NG * P, 8192)
        # owned rows: global i = own0 + 128*s + p ; cols j = 512*jb + jj
        irows = own0 + (np.arange(2)[:, None] * 128
                        + np.arange(P)[None, :]).reshape(2, P)  # [2, P]
        jcols = 512 * jb + np.arange(512)                       # [512]
        wgt = (4.0 * rsc[irows][:, :, None]
               * rsl[jcols][None, None, :])                     # [2, P, 512]
        moq = (0.25 / N * Sc[irows][:, :, None]
               * Sl[jcols][None, None, :])
        # mask the true diagonal
        dmask = irows[:, :, None] == jcols[None, None, :]
        wgt = np.where(dmask, 0.0, wgt)
        wgt_t = np.ascontiguousarray(
            wgt.transpose(1, 0, 2)).astype(ml_dtypes.bfloat16)
        mo_t = np.ascontiguousarray(
            moq.transpose(1, 0, 2)).astype(ml_dtypes.bfloat16)
        maps.append({"xin": np.ascontiguousarray(xina), "wgt": wgt_t,
                     "mo": mo_t})
    return maps


def kernel(x: np.ndarray, _trace: bool = False, **_):
    if "nc" not in _CACHE:
        _CACHE["nc"] = _build()
    nc = _CACHE["nc"]
    res = run_bass_kernel_spmd(nc, _in_maps(x), core_ids=list(range(NCORES)),
                               trace=_trace)
    total = np.float64(0.0)
    for k in range(NCORES):
        total += np.float64(res.results[k]["out"].astype(np.float64).sum())
    _CACHE["last_results"] = res
    return np.asarray(total / (F * (F - 1.0)), dtype=np.float32)


# revision 30
# speedup vs baseline: 1.1143x; 1.0626x over previous
"""Trainium2 Bass kernel for nn_CausalPatternDetector.

Computes mean |corr(x[1:, i], x[:-1, j])| over i != j for x [32768, 1024] f32.

Strategy (8 NeuronCores, 2i x 2j x 2t sharding, fp8 DoubleRow):
  - Host quantizes x to fp8 e4m3 and packs per-core operands in the
    [128, ksub=2, free] DoubleRow layout. Core c = 4*ib + 2*jb + t computes
    the cov partial for i-block [512ib, +512) x j-block [512jb, +512) over
    time-half t. lhsT (xc i-cols, ordered [partner's 256 | own 256], own
    half pre-scaled by 0.25) and rhs (xl j-cols) are merged into one DRAM
    tensor with 8KB rows (16 x 1MB DMAs, 16.8MB/core, ~373 GB/s).
  - Pass 1 (partner's two 128-row chunks) accumulates over all 64 kpairs;
    pass 2 (own chunks) trails (last 5 groups after the RS trigger) so the
    pairwise bf16 ReduceScatter of the pass-1 partial (x0.25, seg-selected
    by a host 0/0.25 flag) can overlap it. recv + own psum = centered cov.
  - Normalization is a pure rescaling (0.1% of FLOPs): the host computes
    exact per-feature sums/sums-of-squares of the fp8 data and ships the
    combined weight tile wgt = 4*mask*rsqrt(nl_j)*rsqrt(nc_i) and the
    mean-outer tile mo/4 per core (bf16, 256KB each). This removes the
    8-way stats AllReduce (the dominant cross-core-skew amplifier); the
    only collective left is the pairwise RS, triggered asynchronously at
    kernel start (its ~38us CC setup runs under the load phase).
  - Tail: |own*0.25 - mo/4 + recv| * (4*wgt) == |cov - mo| * rsqrt-outer,
    abs-reduce -> [128,2]; host sums, scales by 1/(F(F-1)).
"""

import numpy as np
import ml_dtypes

import concourse.bass as bass
import concourse.mybir as mybir
import concourse.tile as tile
from concourse import bacc
from concourse.bass_utils import run_bass_kernel_spmd

P = 128
F = 1024
T = 32768
H = T // 2           # 16384 contraction rows per half
KP = 64              # k-pairs per core (each = 256 rows)
NG = 16              # groups of 4 kpairs
NGI = 11             # pass-2 groups before the RS trigger
NCORES = 8
N = float(T - 1)     # 32767 pair count
F8 = mybir.dt.float8e4
F32 = mybir.dt.float32
BF16 = mybir.dt.bfloat16
NP_F8 = ml_dtypes.float8_e4m3

_CACHE = {}


def _build():
    nc = bacc.Bacc("TRN2", target_bir_lowering=False, debug=False,
                   num_devices=NCORES)

    # merged per-group rows: [lh 4096B | rh 4096B]
    xin = nc.dram_tensor("xin", [NG * P, 8192], F8, kind="ExternalInput")
    wgt_in = nc.dram_tensor("wgt", [P, 2, 512], BF16, kind="ExternalInput")
    mo_in = nc.dram_tensor("mo", [P, 2, 512], BF16, kind="ExternalInput")
    zc = nc.dram_tensor("zc", [P, 2], F32, kind="ExternalInput")
    out = nc.dram_tensor("out", [P, 2], F32, kind="ExternalOutput")

    add = mybir.AluOpType.add
    mult = mybir.AluOpType.mult
    DR = mybir.MatmulPerfMode.DoubleRow

    with tile.TileContext(nc) as tc:
        with (
            tc.tile_pool(name="dram", bufs=1, space="DRAM") as dram,
            tc.tile_pool(name="xp", bufs=1) as xp,
            tc.tile_pool(name="normp", bufs=1) as normp,
            tc.tile_pool(name="psum", bufs=8, space="PSUM") as psum,
        ):
            rs_in = dram.tile([4 * P, 512], BF16)
            rs_out = dram.tile([2 * P, 512], BF16)
            dum_in = dram.tile([2, 512], BF16)
            dum_out = dram.tile([1, 512], BF16)

            # ---- loads: tile [P, 2(lh/rh), 4(ki), 2(ksub), 512] ----
            # bulk loads ride the sync queue's rings only, so the scalar/
            # gpsimd rings stay empty for small latency-critical DMAs
            xg = []

            def load_group(g, split=False):
                t = xp.tile([P, 2, 4, 2, 512], F8, name=f"x{g}", tag=f"x{g}")
                if split:
                    for h in range(2):
                        for ki in range(4):
                            nc.sync.dma_start(
                                t[:, h, ki],
                                xin[P * g: P * g + P,
                                    4096 * h + 1024 * ki:
                                    4096 * h + 1024 * ki + 1024])
                else:
                    nc.sync.dma_start(t[:], xin[P * g: P * g + P, :])
                xg.append(t)

            # warm the CC path: tiny pairwise RS triggered before anything
            # else so the ~13us collective setup runs under the load phase
            dumt = normp.tile([2, 512], BF16)
            nc.gpsimd.memset(dumt[:], 0.0)
            nc.gpsimd.dma_start(dum_in[:], dumt[:])
            nc.gpsimd.collective_compute(
                "ReduceScatter", add,
                replica_groups=[[2 * i, 2 * i + 1] for i in range(4)],
                ins=[dum_in.opt()], outs=[dum_out.opt()])
            load_group(0, split=True)
            ones8 = normp.tile([P, 2, 16], F8)
            nc.gpsimd.memset(ones8[:], 1.0)
            warm = normp.tile([P, 2, 512], F8)
            nc.vector.memset(warm[:], 0.0)
            psw = psum.tile([16, 512], F32, name="psw", tag="ps")
            for i in range(6):
                nc.tensor.matmul(psw[:], ones8[:], warm[:],
                                 start=(i == 0), stop=(i == 5),
                                 perf_mode=DR)
            wgt = normp.tile([P, 2, 512], BF16)
            nc.gpsimd.dma_start(wgt[:], wgt_in[:])
            mo = normp.tile([P, 2, 512], BF16)
            nc.gpsimd.dma_start(mo[:], mo_in[:])
            zc_t = normp.tile([P, 2], F32)
            nc.gpsimd.dma_start(zc_t[:], zc[:])
            for g in range(1, NG):
                load_group(g)

            # ---- matmuls: pass1 = lh cols [0:256] (partner rows, all kp),
            #      pass2 = cols [256:512] (own rows, g<NGI inline) ----
            ps1 = [psum.tile([P, 512], F32, name=f"p1{u}", tag="ps")
                   for u in range(2)]
            ps2 = [psum.tile([P, 512], F32, name=f"p2{u}", tag="ps")
                   for u in range(2)]

            def mm(bank, u, g, ki, kp, base):
                nc.tensor.matmul(
                    bank[:],
                    xg[g][:, 0, ki, :, base + 128 * u: base + 128 * u + 128],
                    xg[g][:, 1, ki],
                    start=(kp == 0), stop=(kp == KP - 1), perf_mode=DR)

            for g in range(NG):
                for ki in range(4):
                    kp = 4 * g + ki
                    for u in range(2):
                        mm(ps1[u], u, g, ki, kp, 0)
                    if g < NGI:
                        for u in range(2):
                            mm(ps2[u], u, g, ki, kp, 256)

            # ---- pass-1 flush (x 0.25/0 seg flags) + bf16 pairwise RS ----
            ot = normp.tile([P, 2, 2, 512], BF16)
            for s in range(2):
                for u in range(2):
                    nc.vector.tensor_scalar(ot[:, s, u], ps1[u][:],
                                            zc_t[:, s: s + 1], None, mult)
                nc.scalar.dma_start(
                    rs_in[256 * s: 256 * s + 256, :].rearrange(
                        "(u p) n -> p u n", p=P), ot[:, s])
            nc.gpsimd.collective_compute(
                "ReduceScatter", add,
                replica_groups=[[2 * i, 2 * i + 1] for i in range(4)],
                ins=[rs_in.opt()], outs=[rs_out.opt()])

            # ---- trailing pass-2 matmuls overlap the RS ----
            for g in range(NGI, NG):
                for ki in range(4):
                    kp = 4 * g + ki
                    for u in range(2):
                        mm(ps2[u], u, g, ki, kp, 256)

            # ---- tail: covm = 0.25*own - mo/4 ; covf = covm + recv ----
            covm = normp.tile([P, 2, 512], BF16)
            for u in range(2):
                nc.vector.tensor_sub(covm[:, u], ps2[u][:], mo[:, u])
            recv = normp.tile([P, 2, 512], BF16)
            nc.scalar.dma_start(
                recv[:], rs_out[:, :].rearrange("(u p) n -> p u n", p=P))
            covf = normp.tile([P, 2, 512], BF16)
            nc.vector.tensor_add(covf[:], covm[:], recv[:])
            nc.vector.tensor_mul(covf[:], covf[:], wgt[:])
            rsum = normp.tile([P, 2], F32)
            nc.vector.tensor_reduce(rsum[:], covf[:], mybir.AxisListType.X,
                                    add, apply_absolute_value=True)
            nc.scalar.dma_start(out[:], rsum[:])

    nc.compile()
    return nc


def _in_maps(x: np.ndarray):
    x8 = np.ascontiguousarray(x, dtype=np.float32).astype(NP_F8)
    xf = x8.astype(np.float32)
    # exact per-feature stats of the fp8 data over the lag windows
    S_all = xf.sum(axis=0)
    Q_all = (xf * xf).sum(axis=0)
    Sl = S_all - xf[T - 1]
    Ql = Q_all - xf[T - 1] * xf[T - 1]
    Sc = S_all - xf[0]
    Qc = Q_all - xf[0] * xf[0]
    nl = Ql - Sl * Sl / N
    ncv = Qc - Sc * Sc / N
    rsl = 1.0 / np.sqrt(nl)
    rsc = 1.0 / np.sqrt(ncv)
    maps = []
    for c in range(NCORES):
        ib, jb, t = c >> 2, (c >> 1) & 1, c & 1
        lo = H * t
        hi = min(lo + H, T - 1)
        n = hi - lo
        xl = np.zeros((H, 512), dtype=NP_F8)
        xl[:n] = x8[lo:hi, 512 * jb: 512 * jb + 512]
        # lh cols: [partner's 256 | own 256 scaled by 0.25]
        xc = np.zeros((H, 512), dtype=NP_F8)
        own0 = 512 * ib + 256 * t
        par0 = 512 * ib + 256 * (1 - t)
        xc[:n, 0:256] = x8[lo + 1: hi + 1, par0: par0 + 256]
        xc[:n, 256:512] = (x8[lo + 1: hi + 1, own0: own0 + 256]
                           .astype(np.float32) * 0.25).astype(NP_F8)
        xl_k = xl.reshape(KP, 2, P, 512)
        xc_k = xc.reshape(KP, 2, P, 512)
        rha = xl_k.reshape(NG, 4, 2, P, 512).transpose(0, 3, 1, 2, 4)
        lha = xc_k.reshape(NG, 4, 2, P, 512).transpose(0, 3, 1, 2, 4)
        xina = np.concatenate(
            [lha.reshape(NG, P, 4096), rha.reshape(NG, P, 4096)],
            axis=2).reshape(NG * P, 8192)
        # owned rows: global i = own0 + 128*s + p ; cols j = 512*jb + jj
        irows = own0 + (np.arange(2)[:, None] * 128
                        + np.arange(P)[None, :]).reshape(2, P)  # [2, P]
        jcols = 512 * jb + np.arange(512)                       # [512]
        wgt = (4.0 * rsc[irows][:, :, None]
               * rsl[jcols][None, None, :])                     # [2, P, 512]
        moq = (0.25 / N * Sc[irows][:, :, None]
               * Sl[jcols][None, None, :])
        # mask the true diagonal
        dmask = irows[:, :, None] == jcols[None, None, :]
        wgt = np.where(dmask, 0.0, wgt)
        wgt_t = np.ascontiguousarray(
            wgt.transpose(1, 0, 2)).astype(ml_dtypes.bfloat16)
        mo_t = np.ascontiguousarray(
            moq.transpose(1, 0, 2)).astype(ml_dtypes.bfloat16)
        zcv = np.zeros((P, 2), dtype=np.float32)
        zcv[:, 1 - t] = 0.25                       # my pass1 -> partner seg
        maps.append({"xin": np.ascontiguousarray(xina), "wgt": wgt_t,
                     "mo": mo_t, "zc": zcv})
    return maps


def kernel(x: np.ndarray, _trace: bool = False, **_):
    if "nc" not in _CACHE:
        _CACHE["nc"] = _build()
    nc = _CACHE["nc"]
    res = run_bass_kernel_spmd(nc, _in_maps(x), core_ids=list(range(NCORES)),
                               trace=_trace)
    total = np.float64(0.0)
    for k in range(NCORES):
        total += np.float64(res.results[k]["out"].astype(np.float64).sum())
    _CACHE["last_results"] = res
    return np.asarray(total / (F * (F - 1.0)), dtype=np.float32)
